# revision 1
# baseline (speedup 1.0000x reference)
"""Trainium2 Bass kernel for causal linear-complexity multi-head attention.

Reference computation (per batch n):
    q = softmax((query @ Wq.T) per-head, axis=Dh)
    k = softmax((key  @ Wk.T) per-head, axis=Dh)
    v = key @ Wv.T
    S[t] = sum_{s<=t} k_s^T v_s          (per-head Dh x Dh running state)
    out[t] = q_t @ S[t]

Sharding: 8 cores = 2 batches x 4 head-groups (4 heads of 64 dims each per
core).  Each core gets host-transposed inputs (d on rows) and computes its
(L x 256) output slice in transposed layout; the host reassembles.

Device algorithm: chunked linear attention, chunk C=256, fp32r matmuls.
Per chunk:
  - project eqT/ekT (transposed layout, exp-ed on ACT) and v (natural),
  - q softmax denominators via ones-matmul (T layout) + reciprocal +
    broadcast-matmul + multiply,
  - k softmax denominators in natural layout (tiny N=2 matmuls) folded into
    v's PSUM->SBUF copy as a per-partition ACT scale (k itself stays
    unnormalized; the scale rides on v),
  - PE-transpose ek to natural layout,
  - per head: masked intra-chunk attention A^T = ek^T q (mask s<=t), the
    inter-chunk term from running state S (SBUF-resident), S update via
    outer-product matmuls on head pairs (diagonal blocks consumed).
"""

import contextlib
import threading
from contextlib import ExitStack

import numpy as np

import concourse.bass as bass
import concourse.mybir as mybir
import concourse.tile as tile
from concourse import bacc
from concourse.bass_utils import run_bass_kernel_spmd

P = 128          # SBUF partitions
D = 1024         # model dim (contraction)
DC = D // P      # d-chunks
J = 256          # per-core output columns (4 heads x 64)
L = 2048         # sequence length
C = 256          # chunk size
NCH = L // C     # chunks
DH = 64          # per-head dim
N_CORES = 8
PRIO_OFF = 40
NSPLIT = 2
BF16_X = False          # projection inputs (x, w) in bf16: halves input DMA

F32 = mybir.dt.float32
F32R = mybir.dt.float32r
BF16 = mybir.dt.bfloat16
EXP = mybir.ActivationFunctionType.Exp
COPY = mybir.ActivationFunctionType.Copy


XDT = BF16 if BF16_X else F32R


def _build_nc():
    nc = bacc.Bacc(trn_type="TRN2", target_bir_lowering=False, num_devices=N_CORES)

    xq = nc.dram_tensor("xq", [D, L], XDT, kind="ExternalInput").ap()
    xk = nc.dram_tensor("xk", [D, L], XDT, kind="ExternalInput").ap()
    wq = nc.dram_tensor("wq", [D, J], XDT, kind="ExternalInput").ap()
    wk = nc.dram_tensor("wk", [D, J], XDT, kind="ExternalInput").ap()
    wv = nc.dram_tensor("wv", [D, J], XDT, kind="ExternalInput").ap()
    mask = nc.dram_tensor("mask", [P, 2, C], F32R, kind="ExternalInput").ap()
    emap = nc.dram_tensor("emap", [2, P], F32R, kind="ExternalInput").ap()
    eones = nc.dram_tensor("eones", [P, 2], F32R, kind="ExternalInput").ap()
    ident = nc.dram_tensor("ident", [P, P], F32R, kind="ExternalInput").ap()
    outT = nc.dram_tensor("outT", [J, L], F32, kind="ExternalOutput").ap()

    xq_r = xq.rearrange("(dc p) t -> p dc t", p=P)
    xk_r = xk.rearrange("(dc p) t -> p dc t", p=P)
    wq_r = wq.rearrange("(dc p) j -> p dc j", p=P)
    wk_r = wk.rearrange("(dc p) j -> p dc j", p=P)
    wv_r = wv.rearrange("(dc p) j -> p dc j", p=P)
    outT_r = outT.rearrange("(jt p) t -> p jt t", p=P)

    with tile.TileContext(nc) as tc, ExitStack() as ctx:
        ctx.enter_context(
            nc.allow_low_precision(reason="float32r tiles carry fp32 bits")
        )
        cpool = ctx.enter_context(tc.tile_pool(name="consts", bufs=1))
        xpool = ctx.enter_context(tc.tile_pool(name="xin", bufs=5))
        spool = ctx.enter_context(tc.tile_pool(name="sb", bufs=3))
        ppool = ctx.enter_context(tc.tile_pool(name="pp", bufs=3, space="PSUM"))
        pnorm = ctx.enter_context(tc.tile_pool(name="pn", bufs=1, space="PSUM"))
        patp = ctx.enter_context(tc.tile_pool(name="pa", bufs=2, space="PSUM"))
        potp = ctx.enter_context(tc.tile_pool(name="po", bufs=2, space="PSUM"))

        # ---- constants / persistent state ----
        # DMA issue order = SP-ring FIFO order: chunk-0 activations and the
        # weights they need, interleaved in dc-halves so the first projection
        # matmuls can start after ~1MB instead of ~5MB; small constants after.
        HDC = DC // NSPLIT
        xq_h0 = [xpool.tile([P, HDC, C], XDT, tag=f"xq_h{i}", name=f"xq_h{i}") for i in range(NSPLIT)]
        xk_h0 = [xpool.tile([P, HDC, C], XDT, tag=f"xk_h{i}", name=f"xk_h{i}") for i in range(NSPLIT)]
        wq_h = [cpool.tile([P, HDC, J], XDT, tag=f"wq_h{i}", name=f"wq_h{i}") for i in range(NSPLIT)]
        wk_h = [cpool.tile([P, HDC, J], XDT, tag=f"wk_h{i}", name=f"wk_h{i}") for i in range(NSPLIT)]
        wv_h = [cpool.tile([P, HDC, J], XDT, tag=f"wv_h{i}", name=f"wv_h{i}") for i in range(NSPLIT)]
        for i in range(NSPLIT):
            dcs = slice(i * HDC, (i + 1) * HDC)
            nc.sync.dma_start(xq_h0[i][:], xq_r[:, dcs, 0:C])
            nc.sync.dma_start(wq_h[i][:], wq_r[:, dcs, :])
        for i in range(NSPLIT):
            dcs = slice(i * HDC, (i + 1) * HDC)
            nc.sync.dma_start(xk_h0[i][:], xk_r[:, dcs, 0:C])
            nc.sync.dma_start(wk_h[i][:], wk_r[:, dcs, :])
        for i in range(NSPLIT):
            dcs = slice(i * HDC, (i + 1) * HDC)
            nc.sync.dma_start(wv_h[i][:], wv_r[:, dcs, :])
        eones_sb = cpool.tile([P, 2], F32R, tag="eones_sb")
        nc.sync.dma_start(eones_sb[:], eones[:])
        emap_sb = cpool.tile([2, P], F32R, tag="emap_sb")
        nc.sync.dma_start(emap_sb[:], emap[:])
        ident_sb = cpool.tile([P, P], F32R, tag="ident_sb")
        nc.sync.dma_start(ident_sb[:], ident[:])
        mask_sb = cpool.tile([P, 2, C], F32R, tag="mask_sb")
        nc.sync.dma_start(mask_sb[:], mask[:])
        S_sb = cpool.tile([P, 2, DH], F32R, tag="S_sb")
        # ping-pong masked-attention tiles; [:, 1, 0:128] is always zero
        # (s-tile 1 never attends to the first t-half of a chunk)
        at_mt = []
        for i in range(4):
            t = cpool.tile([P, 2, C], F32R, tag=f"at_m{i}")
            nc.vector.memset(t[:, 1, 0:P].bitcast(F32), 0.0)
            at_mt.append(t)

        for c in range(NCH):
            ts = slice(c * C, (c + 1) * C)

            if c == 0:
                xq_h, xk_h = xq_h0, xk_h0
            else:
                with tc.high_priority(offset=PRIO_OFF):
                    xq_h = [xpool.tile([P, HDC, C], XDT, tag=f"xq_h{i}",
                                       name=f"xq_h{i}") for i in range(NSPLIT)]
                    xk_h = [xpool.tile([P, HDC, C], XDT, tag=f"xk_h{i}",
                                       name=f"xk_h{i}") for i in range(NSPLIT)]
                    for i in range(NSPLIT):
                        dcs = slice(i * HDC, (i + 1) * HDC)
                        nc.sync.dma_start(xq_h[i][:], xq_r[:, dcs, ts])
                        nc.sync.dma_start(xk_h[i][:], xk_r[:, dcs, ts])

            # ---- q/k projections (transposed layout) + exp ----
            prio = (tc.high_priority(offset=PRIO_OFF)
                    if c > 0 else contextlib.nullcontext())
            with prio:
                eq_e = spool.tile([P, 2, C], F32R, tag="eq_e")
                ek_e = spool.tile([P, 2, C], F32R, tag="ek_e")
                for w_h, x_h, e_t, ptag in (
                    (wq_h, xq_h, eq_e, "pq"),
                    (wk_h, xk_h, ek_e, "pk"),
                ):
                    pe_t = ppool.tile([P, 2, C], F32, tag="pp")
                    for jt in range(2):
                        for dc in range(DC):
                            i, d = dc // HDC, dc % HDC
                            nc.tensor.matmul(
                                pe_t[:, jt, :],
                                w_h[i][:, d, jt * P:(jt + 1) * P],
                                x_h[i][:, d, :],
                                start=(dc == 0),
                                stop=(dc == DC - 1),
                            )
                    nc.scalar.activation(e_t[:], pe_t[:], EXP)

                # ---- v projection (natural layout) ----
                pv_t = ppool.tile([P, 2, J], F32, tag="pp")
                for st in range(2):
                    for dc in range(DC):
                        i, d = dc // HDC, dc % HDC
                        nc.tensor.matmul(
                            pv_t[:, st, :],
                            xk_h[i][:, d, st * P:(st + 1) * P],
                            wv_h[i][:, d, :],
                            start=(dc == 0),
                            stop=(dc == DC - 1),
                        )

            # ---- k denominators (natural layout) -> v scale ----
            pdk = pnorm.tile([P, 2, 2, 2], F32, tag="pn")
            for jt in range(2):
                for th in range(2):
                    nc.tensor.matmul(
                        pdk[:, jt, th, :],
                        ek_e[:, jt, th * P:(th + 1) * P],
                        eones_sb[:],
                        start=True,
                        stop=True,
                    )
            rk = spool.tile([P, 2, 2, 2], F32, tag="rk")
            nc.vector.reciprocal(rk[:], pdk[:])
            v_sb = spool.tile([P, 2, J], F32R, tag="v_sb")
            for st in range(2):
                for jt in range(2):
                    for half in range(2):
                        h = 2 * jt + half
                        nc.scalar.activation(
                            v_sb[:, st, h * DH:(h + 1) * DH],
                            pv_t[:, st, h * DH:(h + 1) * DH],
                            COPY,
                            scale=rk[:, jt, st, half:half + 1],
                        )

            # ---- transpose ek to natural layout (not needed after the
            # last state update) ----
            if c < NCH - 1:
                ekn_sb = spool.tile([P, 2, J], F32R, tag="ekn_sb")
                for jt in range(2):
                    for st in range(2):
                        ptr = patp.tile([P, P], F32R, tag="pat")
                        nc.tensor.transpose(
                            ptr[:], ek_e[:, jt, st * P:(st + 1) * P],
                            ident_sb[:]
                        )
                        nc.vector.tensor_copy(
                            ekn_sb[:, st, jt * P:(jt + 1) * P], ptr[:]
                        )

            # ---- q denominators (T layout) + normalization ----
            pnq = pnorm.tile([P, 2, C], F32, tag="pn")
            nc.tensor.matmul(
                pnq[0:2, :, :], eones_sb[:], eq_e[:], start=True, stop=True
            )
            rq = spool.tile([2, 2, C], F32R, tag="rq")
            nc.vector.reciprocal(rq[:], pnq[0:2, :, :])
            nc.tensor.matmul(pnq[:], emap_sb[:], rq[:], start=True, stop=True)
            # rqb[j, t] = 1/dq[head(j), t] — same layout as outT, so the
            # q-softmax normalization is applied on the output copy instead
            # of on eq (keeps the denominator chain off the AT-matmul path)
            rqb_sb = spool.tile([P, 2, C], F32, tag="rqb_sb")
            nc.vector.tensor_copy(rqb_sb[:], pnq[:])

            # ---- per-head attention: intra (masked) + inter (state) ----
            oc = spool.tile([P, 2, C], F32, tag="oc")
            for jt in range(2):
                pats = []
                for half in range(2):
                    rows = slice(64 * half, 64 * half + 64)
                    pat = patp.tile([P, 2, C], F32, tag="pat")
                    for st in range(2):
                        nc.tensor.matmul(
                            pat[:, st, :],
                            ek_e[rows, jt, st * P:(st + 1) * P],
                            eq_e[rows, jt, :],
                            start=True,
                            stop=True,
                        )
                    pats.append(pat)
                for half in range(2):
                    h = 2 * jt + half
                    rows = slice(64 * half, 64 * half + 64)
                    pat = pats[half]
                    at_m = at_mt[2 * (c % 2) + half]
                    nc.vector.tensor_mul(
                        at_m[:, 0, :], pat[:, 0, :], mask_sb[:, 0, :]
                    )
                    nc.vector.tensor_mul(
                        at_m[:, 1, P:], pat[:, 1, P:], mask_sb[:, 1, P:]
                    )
                    pot = potp.tile([64, C], F32, tag="pot")
                    nc.tensor.matmul(
                        pot[:],
                        v_sb[:, 0, h * DH:(h + 1) * DH],
                        at_m[:, 0, :],
                        start=True,
                        stop=False,
                    )
                    nc.tensor.matmul(
                        pot[:],
                        v_sb[:, 1, h * DH:(h + 1) * DH],
                        at_m[:, 1, :],
                        start=False,
                        stop=(c == 0),
                    )
                    if c > 0:
                        nc.tensor.matmul(
                            pot[:],
                            S_sb[rows, jt, :],
                            eq_e[rows, jt, :],
                            start=False,
                            stop=True,
                        )
                    nc.vector.tensor_mul(
                        oc[rows, jt, :], pot[:], rqb_sb[rows, jt, :]
                    )

            # ---- state update: S += ek_nat^T v (diagonal head blocks) ----
            for jt in range(2):
                if c == NCH - 1:
                    break
                # full-width rhs keeps the fp32r matmul at 1 cycle/row
                # (N=256); the off-pair half of the output is unused.
                pds = patp.tile([P, J], F32, tag="pat")
                for st in range(2):
                    nc.tensor.matmul(
                        pds[:],
                        ekn_sb[:, st, jt * P:(jt + 1) * P],
                        v_sb[:, st, :],
                        start=(st == 0),
                        stop=(st == 1),
                    )
                for half in range(2):
                    rows = slice(64 * half, 64 * half + 64)
                    col = jt * P + 64 * half
                    if c == 0:
                        nc.vector.tensor_copy(
                            S_sb[rows, jt, :], pds[rows, col:col + 64]
                        )
                    else:
                        nc.vector.tensor_add(
                            S_sb[rows, jt, :],
                            S_sb[rows, jt, :],
                            pds[rows, col:col + 64],
                        )

            # output store on the ACT HWDGE ring: keeps the SP ring free for
            # input prefetch (FIFO per ring; a waiting store would otherwise
            # block the next chunk's loads)
            for jt in range(2):
                nc.scalar.dma_start(outT_r[:, jt, ts], oc[:, jt, :])

    nc.finalize()
    return nc


def _host_inputs(query, key, Wq, Wk, Wv):
    """Build the 8 per-core input maps (host-side layout prep)."""
    import ml_dtypes
    xdt = ml_dtypes.bfloat16 if BF16_X else np.float32
    mask = np.zeros((P, 2, C), np.float32)
    for st in range(2):
        s = st * P + np.arange(P)[:, None]
        t = np.arange(C)[None, :]
        mask[:, st, :] = (s <= t).astype(np.float32)
    emap = np.zeros((2, P), np.float32)
    emap[0, :64] = 1.0
    emap[1, 64:] = 1.0
    eones = np.zeros((P, 2), np.float32)
    eones[:64, 0] = 1.0
    eones[64:, 1] = 1.0
    ident = np.eye(P, dtype=np.float32)

    per_batch = {}
    for n in range(2):
        per_batch[n] = (
            np.ascontiguousarray(query[n].T).astype(xdt),
            np.ascontiguousarray(key[n].T).astype(xdt),
        )

    in_maps = []
    for core in range(N_CORES):
        n, g = core // 4, core % 4
        xq, xk = per_batch[n]
        cols = slice(g * J, (g + 1) * J)
        in_maps.append({
            "xq": xq,
            "xk": xk,
            "wq": np.ascontiguousarray(Wq[cols, :].T).astype(xdt),
            "wk": np.ascontiguousarray(Wk[cols, :].T).astype(xdt),
            "wv": np.ascontiguousarray(Wv[cols, :].T).astype(xdt),
            "mask": mask,
            "emap": emap,
            "eones": eones,
            "ident": ident,
        })
    return in_maps


_NC_LOCK = threading.Lock()
_NC_CACHE = {}


def _get_nc():
    with _NC_LOCK:
        if "nc" not in _NC_CACHE:
            _NC_CACHE["nc"] = _build_nc()
        return _NC_CACHE["nc"]


def kernel(query, key, Wq, Wk, Wv, _trace=False, _trace_kwargs=None):
    query = np.asarray(query)
    key = np.asarray(key)
    Wq = np.asarray(Wq)
    Wk = np.asarray(Wk)
    Wv = np.asarray(Wv)

    nc = _get_nc()
    in_maps = _host_inputs(query, key, Wq, Wk, Wv)
    res = run_bass_kernel_spmd(
        nc, in_maps, core_ids=list(range(N_CORES)),
        trace=_trace, **(_trace_kwargs or {}),
    )

    out = np.empty((2, L, D), np.float32)
    for core, r in enumerate(res.results):
        n, g = core // 4, core % 4
        out[n, :, g * J:(g + 1) * J] = r["outT"].T
    if _trace:
        kernel.last_results = res
    return out



# revision 2
# speedup vs baseline: 1.0704x; 1.0704x over previous
"""Trainium2 Bass kernel for causal linear-complexity multi-head attention.

Reference computation (per batch n):
    q = softmax((query @ Wq.T) per-head, axis=Dh)
    k = softmax((key  @ Wk.T) per-head, axis=Dh)
    v = key @ Wv.T
    S[t] = sum_{s<=t} k_s^T v_s          (per-head Dh x Dh running state)
    out[t] = q_t @ S[t]

Sharding: 8 cores = 2 batches x 4 head-groups (4 heads of 64 dims each per
core).  Each core gets host-transposed inputs (d on rows) and computes its
(L x 256) output slice in transposed layout; the host reassembles.

Device algorithm: chunked linear attention, chunk C=256, fp32r matmuls.
Per chunk:
  - project eqT/ekT (transposed layout, exp-ed on ACT) and v (natural),
  - q softmax denominators via ones-matmul (T layout) + reciprocal +
    broadcast-matmul + multiply,
  - k softmax denominators in natural layout (tiny N=2 matmuls) folded into
    v's PSUM->SBUF copy as a per-partition ACT scale (k itself stays
    unnormalized; the scale rides on v),
  - PE-transpose ek to natural layout,
  - per head: masked intra-chunk attention A^T = ek^T q (mask s<=t), the
    inter-chunk term from running state S (SBUF-resident), S update via
    outer-product matmuls on head pairs (diagonal blocks consumed).
"""

import contextlib
import threading
from contextlib import ExitStack

import numpy as np

import concourse.bass as bass
import concourse.mybir as mybir
import concourse.tile as tile
from concourse import bacc
from concourse.bass_utils import run_bass_kernel_spmd

P = 128          # SBUF partitions
D = 1024         # model dim (contraction)
DC = D // P      # d-chunks
J = 256          # per-core output columns (4 heads x 64)
L = 2048         # sequence length
C = 256          # chunk size
NCH = L // C     # chunks
DH = 64          # per-head dim
N_CORES = 8
PRIO_OFF = 40
NSPLIT = 2
BF16_X = True          # projection inputs (x, w) in bf16: halves input DMA

F32 = mybir.dt.float32
F32R = mybir.dt.float32r
BF16 = mybir.dt.bfloat16
EXP = mybir.ActivationFunctionType.Exp
COPY = mybir.ActivationFunctionType.Copy


XDT = BF16 if BF16_X else F32R


def _build_nc():
    nc = bacc.Bacc(trn_type="TRN2", target_bir_lowering=False, num_devices=N_CORES)

    xq = nc.dram_tensor("xq", [D, L], XDT, kind="ExternalInput").ap()
    xk = nc.dram_tensor("xk", [D, L], XDT, kind="ExternalInput").ap()
    wq = nc.dram_tensor("wq", [D, J], XDT, kind="ExternalInput").ap()
    wk = nc.dram_tensor("wk", [D, J], XDT, kind="ExternalInput").ap()
    wv = nc.dram_tensor("wv", [D, J], XDT, kind="ExternalInput").ap()
    mask = nc.dram_tensor("mask", [P, 2, C], F32R, kind="ExternalInput").ap()
    emap = nc.dram_tensor("emap", [2, P], F32R, kind="ExternalInput").ap()
    eones = nc.dram_tensor("eones", [P, 2], F32R, kind="ExternalInput").ap()
    ident = nc.dram_tensor("ident", [P, P], F32R, kind="ExternalInput").ap()
    outT = nc.dram_tensor("outT", [J, L], F32, kind="ExternalOutput").ap()

    xq_r = xq.rearrange("(dc p) t -> p dc t", p=P)
    xk_r = xk.rearrange("(dc p) t -> p dc t", p=P)
    wq_r = wq.rearrange("(dc p) j -> p dc j", p=P)
    wk_r = wk.rearrange("(dc p) j -> p dc j", p=P)
    wv_r = wv.rearrange("(dc p) j -> p dc j", p=P)
    outT_r = outT.rearrange("(jt p) t -> p jt t", p=P)

    with tile.TileContext(nc) as tc, ExitStack() as ctx:
        ctx.enter_context(
            nc.allow_low_precision(reason="float32r tiles carry fp32 bits")
        )
        cpool = ctx.enter_context(tc.tile_pool(name="consts", bufs=1))
        xpool = ctx.enter_context(tc.tile_pool(name="xin", bufs=5))
        spool = ctx.enter_context(tc.tile_pool(name="sb", bufs=3))
        ppool = ctx.enter_context(tc.tile_pool(name="pp", bufs=3, space="PSUM"))
        pnorm = ctx.enter_context(tc.tile_pool(name="pn", bufs=1, space="PSUM"))
        patp = ctx.enter_context(tc.tile_pool(name="pa", bufs=2, space="PSUM"))
        potp = ctx.enter_context(tc.tile_pool(name="po", bufs=2, space="PSUM"))

        # ---- constants / persistent state ----
        # DMA issue order = SP-ring FIFO order: chunk-0 activations and the
        # weights they need, interleaved in dc-halves so the first projection
        # matmuls can start after ~1MB instead of ~5MB; small constants after.
        HDC = DC // NSPLIT
        xq_h0 = [xpool.tile([P, HDC, C], XDT, tag=f"xq_h{i}", name=f"xq_h{i}") for i in range(NSPLIT)]
        xk_h0 = [xpool.tile([P, HDC, C], XDT, tag=f"xk_h{i}", name=f"xk_h{i}") for i in range(NSPLIT)]
        wq_h = [cpool.tile([P, HDC, J], XDT, tag=f"wq_h{i}", name=f"wq_h{i}") for i in range(NSPLIT)]
        wk_h = [cpool.tile([P, HDC, J], XDT, tag=f"wk_h{i}", name=f"wk_h{i}") for i in range(NSPLIT)]
        wv_h = [cpool.tile([P, HDC, J], XDT, tag=f"wv_h{i}", name=f"wv_h{i}") for i in range(NSPLIT)]
        for i in range(NSPLIT):
            dcs = slice(i * HDC, (i + 1) * HDC)
            nc.sync.dma_start(xq_h0[i][:], xq_r[:, dcs, 0:C])
            nc.sync.dma_start(wq_h[i][:], wq_r[:, dcs, :])
        for i in range(NSPLIT):
            dcs = slice(i * HDC, (i + 1) * HDC)
            nc.sync.dma_start(xk_h0[i][:], xk_r[:, dcs, 0:C])
            nc.sync.dma_start(wk_h[i][:], wk_r[:, dcs, :])
        for i in range(NSPLIT):
            dcs = slice(i * HDC, (i + 1) * HDC)
            nc.sync.dma_start(wv_h[i][:], wv_r[:, dcs, :])
        eones_sb = cpool.tile([P, 2], F32R, tag="eones_sb")
        nc.sync.dma_start(eones_sb[:], eones[:])
        emap_sb = cpool.tile([2, P], F32R, tag="emap_sb")
        nc.sync.dma_start(emap_sb[:], emap[:])
        ident_sb = cpool.tile([P, P], F32R, tag="ident_sb")
        nc.sync.dma_start(ident_sb[:], ident[:])
        mask_sb = cpool.tile([P, 2, C], F32R, tag="mask_sb")
        nc.sync.dma_start(mask_sb[:], mask[:])
        S_sb = cpool.tile([P, 2, DH], F32R, tag="S_sb")
        # ping-pong masked-attention tiles; [:, 1, 0:128] is always zero
        # (s-tile 1 never attends to the first t-half of a chunk)
        at_mt = []
        for i in range(4):
            t = cpool.tile([P, 2, C], F32R, tag=f"at_m{i}")
            nc.vector.memset(t[:, 1, 0:P].bitcast(F32), 0.0)
            at_mt.append(t)

        for c in range(NCH):
            ts = slice(c * C, (c + 1) * C)

            if c == 0:
                xq_h, xk_h = xq_h0, xk_h0
            else:
                with tc.high_priority(offset=PRIO_OFF):
                    xq_h = [xpool.tile([P, HDC, C], XDT, tag=f"xq_h{i}",
                                       name=f"xq_h{i}") for i in range(NSPLIT)]
                    xk_h = [xpool.tile([P, HDC, C], XDT, tag=f"xk_h{i}",
                                       name=f"xk_h{i}") for i in range(NSPLIT)]
                    for i in range(NSPLIT):
                        dcs = slice(i * HDC, (i + 1) * HDC)
                        nc.sync.dma_start(xq_h[i][:], xq_r[:, dcs, ts])
                        nc.sync.dma_start(xk_h[i][:], xk_r[:, dcs, ts])

            # ---- q/k projections (transposed layout) + exp ----
            prio = (tc.high_priority(offset=PRIO_OFF)
                    if c > 0 else contextlib.nullcontext())
            with prio:
                eq_e = spool.tile([P, 2, C], F32R, tag="eq_e")
                ek_e = spool.tile([P, 2, C], F32R, tag="ek_e")
                for w_h, x_h, e_t, ptag in (
                    (wq_h, xq_h, eq_e, "pq"),
                    (wk_h, xk_h, ek_e, "pk"),
                ):
                    pe_t = ppool.tile([P, 2, C], F32, tag="pp")
                    for jt in range(2):
                        for dc in range(DC):
                            i, d = dc // HDC, dc % HDC
                            nc.tensor.matmul(
                                pe_t[:, jt, :],
                                w_h[i][:, d, jt * P:(jt + 1) * P],
                                x_h[i][:, d, :],
                                start=(dc == 0),
                                stop=(dc == DC - 1),
                            )
                    nc.scalar.activation(e_t[:], pe_t[:], EXP)

                # ---- v projection (natural layout) ----
                pv_t = ppool.tile([P, 2, J], F32, tag="pp")
                for st in range(2):
                    for dc in range(DC):
                        i, d = dc // HDC, dc % HDC
                        nc.tensor.matmul(
                            pv_t[:, st, :],
                            xk_h[i][:, d, st * P:(st + 1) * P],
                            wv_h[i][:, d, :],
                            start=(dc == 0),
                            stop=(dc == DC - 1),
                        )

            # ---- k denominators (natural layout) -> v scale ----
            pdk = pnorm.tile([P, 2, 2, 2], F32, tag="pn")
            for jt in range(2):
                for th in range(2):
                    nc.tensor.matmul(
                        pdk[:, jt, th, :],
                        ek_e[:, jt, th * P:(th + 1) * P],
                        eones_sb[:],
                        start=True,
                        stop=True,
                    )
            rk = spool.tile([P, 2, 2, 2], F32, tag="rk")
            nc.vector.reciprocal(rk[:], pdk[:])
            v_sb = spool.tile([P, 2, J], F32R, tag="v_sb")
            for st in range(2):
                for jt in range(2):
                    for half in range(2):
                        h = 2 * jt + half
                        nc.scalar.activation(
                            v_sb[:, st, h * DH:(h + 1) * DH],
                            pv_t[:, st, h * DH:(h + 1) * DH],
                            COPY,
                            scale=rk[:, jt, st, half:half + 1],
                        )

            # ---- transpose ek to natural layout (not needed after the
            # last state update) ----
            if c < NCH - 1:
                ekn_sb = spool.tile([P, 2, J], F32R, tag="ekn_sb")
                for jt in range(2):
                    for st in range(2):
                        ptr = patp.tile([P, P], F32R, tag="pat")
                        nc.tensor.transpose(
                            ptr[:], ek_e[:, jt, st * P:(st + 1) * P],
                            ident_sb[:]
                        )
                        nc.vector.tensor_copy(
                            ekn_sb[:, st, jt * P:(jt + 1) * P], ptr[:]
                        )

            # ---- q denominators (T layout) + normalization ----
            pnq = pnorm.tile([P, 2, C], F32, tag="pn")
            nc.tensor.matmul(
                pnq[0:2, :, :], eones_sb[:], eq_e[:], start=True, stop=True
            )
            rq = spool.tile([2, 2, C], F32R, tag="rq")
            nc.vector.reciprocal(rq[:], pnq[0:2, :, :])
            nc.tensor.matmul(pnq[:], emap_sb[:], rq[:], start=True, stop=True)
            # rqb[j, t] = 1/dq[head(j), t] — same layout as outT, so the
            # q-softmax normalization is applied on the output copy instead
            # of on eq (keeps the denominator chain off the AT-matmul path)
            rqb_sb = spool.tile([P, 2, C], F32, tag="rqb_sb")
            nc.vector.tensor_copy(rqb_sb[:], pnq[:])

            # ---- per-head attention: intra (masked) + inter (state) ----
            oc = spool.tile([P, 2, C], F32, tag="oc")
            for jt in range(2):
                pats = []
                for half in range(2):
                    rows = slice(64 * half, 64 * half + 64)
                    pat = patp.tile([P, 2, C], F32, tag="pat")
                    for st in range(2):
                        nc.tensor.matmul(
                            pat[:, st, :],
                            ek_e[rows, jt, st * P:(st + 1) * P],
                            eq_e[rows, jt, :],
                            start=True,
                            stop=True,
                        )
                    pats.append(pat)
                for half in range(2):
                    h = 2 * jt + half
                    rows = slice(64 * half, 64 * half + 64)
                    pat = pats[half]
                    at_m = at_mt[2 * (c % 2) + half]
                    nc.vector.tensor_mul(
                        at_m[:, 0, :], pat[:, 0, :], mask_sb[:, 0, :]
                    )
                    nc.vector.tensor_mul(
                        at_m[:, 1, P:], pat[:, 1, P:], mask_sb[:, 1, P:]
                    )
                    pot = potp.tile([64, C], F32, tag="pot")
                    nc.tensor.matmul(
                        pot[:],
                        v_sb[:, 0, h * DH:(h + 1) * DH],
                        at_m[:, 0, :],
                        start=True,
                        stop=False,
                    )
                    nc.tensor.matmul(
                        pot[:],
                        v_sb[:, 1, h * DH:(h + 1) * DH],
                        at_m[:, 1, :],
                        start=False,
                        stop=(c == 0),
                    )
                    if c > 0:
                        nc.tensor.matmul(
                            pot[:],
                            S_sb[rows, jt, :],
                            eq_e[rows, jt, :],
                            start=False,
                            stop=True,
                        )
                    nc.vector.tensor_mul(
                        oc[rows, jt, :], pot[:], rqb_sb[rows, jt, :]
                    )

            # ---- state update: S += ek_nat^T v (diagonal head blocks) ----
            for jt in range(2):
                if c == NCH - 1:
                    break
                # full-width rhs keeps the fp32r matmul at 1 cycle/row
                # (N=256); the off-pair half of the output is unused.
                pds = patp.tile([P, J], F32, tag="pat")
                for st in range(2):
                    nc.tensor.matmul(
                        pds[:],
                        ekn_sb[:, st, jt * P:(jt + 1) * P],
                        v_sb[:, st, :],
                        start=(st == 0),
                        stop=(st == 1),
                    )
                for half in range(2):
                    rows = slice(64 * half, 64 * half + 64)
                    col = jt * P + 64 * half
                    if c == 0:
                        nc.vector.tensor_copy(
                            S_sb[rows, jt, :], pds[rows, col:col + 64]
                        )
                    else:
                        nc.vector.tensor_add(
                            S_sb[rows, jt, :],
                            S_sb[rows, jt, :],
                            pds[rows, col:col + 64],
                        )

            # output store on the ACT HWDGE ring: keeps the SP ring free for
            # input prefetch (FIFO per ring; a waiting store would otherwise
            # block the next chunk's loads)
            for jt in range(2):
                nc.scalar.dma_start(outT_r[:, jt, ts], oc[:, jt, :])

    nc.finalize()
    return nc


def _host_inputs(query, key, Wq, Wk, Wv):
    """Build the 8 per-core input maps (host-side layout prep)."""
    import ml_dtypes
    xdt = ml_dtypes.bfloat16 if BF16_X else np.float32
    mask = np.zeros((P, 2, C), np.float32)
    for st in range(2):
        s = st * P + np.arange(P)[:, None]
        t = np.arange(C)[None, :]
        mask[:, st, :] = (s <= t).astype(np.float32)
    emap = np.zeros((2, P), np.float32)
    emap[0, :64] = 1.0
    emap[1, 64:] = 1.0
    eones = np.zeros((P, 2), np.float32)
    eones[:64, 0] = 1.0
    eones[64:, 1] = 1.0
    ident = np.eye(P, dtype=np.float32)

    per_batch = {}
    for n in range(2):
        per_batch[n] = (
            np.ascontiguousarray(query[n].T).astype(xdt),
            np.ascontiguousarray(key[n].T).astype(xdt),
        )

    in_maps = []
    for core in range(N_CORES):
        n, g = core // 4, core % 4
        xq, xk = per_batch[n]
        cols = slice(g * J, (g + 1) * J)
        in_maps.append({
            "xq": xq,
            "xk": xk,
            "wq": np.ascontiguousarray(Wq[cols, :].T).astype(xdt),
            "wk": np.ascontiguousarray(Wk[cols, :].T).astype(xdt),
            "wv": np.ascontiguousarray(Wv[cols, :].T).astype(xdt),
            "mask": mask,
            "emap": emap,
            "eones": eones,
            "ident": ident,
        })
    return in_maps


_NC_LOCK = threading.Lock()
_NC_CACHE = {}


def _get_nc():
    with _NC_LOCK:
        if "nc" not in _NC_CACHE:
            _NC_CACHE["nc"] = _build_nc()
        return _NC_CACHE["nc"]


def kernel(query, key, Wq, Wk, Wv, _trace=False, _trace_kwargs=None):
    query = np.asarray(query)
    key = np.asarray(key)
    Wq = np.asarray(Wq)
    Wk = np.asarray(Wk)
    Wv = np.asarray(Wv)

    nc = _get_nc()
    in_maps = _host_inputs(query, key, Wq, Wk, Wv)
    res = run_bass_kernel_spmd(
        nc, in_maps, core_ids=list(range(N_CORES)),
        trace=_trace, **(_trace_kwargs or {}),
    )

    out = np.empty((2, L, D), np.float32)
    for core, r in enumerate(res.results):
        n, g = core // 4, core % 4
        out[n, :, g * J:(g + 1) * J] = r["outT"].T
    if _trace:
        kernel.last_results = res
    return out

